# revision 1
# baseline (speedup 1.0000x reference)
"""MoE text projection kernel for 8 TRN2 NeuronCores (Bass/Tile).

Problem: x[32,1024,768], gate_W[768,8], gate_b[8], expert_W[8,768,256],
expert_b[8,256] -> out[32,1024,256].  top-2 of 8 experts, softmax-over-all
gate, dense all-expert projection with masked weighted combine.

Strategy: data-parallel over tokens (32768 tokens -> 4096/core).  Host
pre-transposes x to xT[768, 4096] per core (contraction dim on partitions)
and rearranges expert_W; weights replicated.  On device per core:
  - gate logits in exact fp32 (top-2 selection is numerically sensitive),
  - softmax + top-2 mask via Max8 on VectorE,
  - all-8-expert projections in float32r (TF32-ish, 1 cyc/row) with PSUM
    accumulation over the 768-contraction,
  - weighted combine via per-partition-scalar fused multiply-add on VectorE,
  - expert-bias term via a tiny K=8 matmul (wm^T @ expert_b).
No collectives: outputs are disjoint token shards, host concatenates.
"""
import sys

sys.path.insert(0, "/opt/trn_rl_repo")

import numpy as np

# hardcoded problem shapes
BS, L, DIN, DOUT, E = 32, 1024, 768, 256, 8
NCORES = 8
NTOK = BS * L              # 32768
T = NTOK // NCORES         # 4096 tokens per core
KC = DIN // 128            # 6 contraction chunks
NG = 8                     # groups per core
TG = T // NG               # 512 tokens per group
NT = TG // 128             # 4 tiles per group

_STATE: dict = {}


def _build_program(reps: int = 1, use_act_round: bool = True,
                   expert_dtype: str = "f32r", dma_engine: str = "sync"):
    import concourse.mybir as mybir
    from concourse import bacc
    from concourse.tile import TileContext
    from concourse.masks import make_identity

    f32 = mybir.dt.float32
    f32r = (mybir.dt.float32r if expert_dtype == "f32r"
            else mybir.dt.bfloat16)

    nc = bacc.Bacc("TRN2", target_bir_lowering=False, debug=False,
                   num_devices=NCORES)
    xT_d = nc.dram_tensor("xt", [DIN, T], f32, kind="ExternalInput")
    gw_d = nc.dram_tensor("gw", [128, KC * E], f32, kind="ExternalInput")
    gb_d = nc.dram_tensor("gb", [128, NT * E], f32, kind="ExternalInput")
    ew_d = nc.dram_tensor("ew", [128, KC * E * DOUT], f32, kind="ExternalInput")
    eb_d = nc.dram_tensor("eb", [E, DOUT], f32, kind="ExternalInput")
    out_d = nc.dram_tensor("out", [T, DOUT], f32, kind="ExternalOutput")

    AL = mybir.AluOpType
    AF = mybir.ActivationFunctionType
    dma = nc.sync if dma_engine == "sync" else nc.gpsimd

    with TileContext(nc) as tc:
        with (
            tc.tile_pool(name="const", bufs=1) as cpool,
            tc.tile_pool(name="xg", bufs=2) as xg_pool,
            tc.tile_pool(name="xgr", bufs=2) as xgr_pool,
            tc.tile_pool(name="sm", bufs=4) as sm,
            tc.tile_pool(name="wm", bufs=2) as wm_pool,
            tc.tile_pool(name="wmt", bufs=2) as wmt_pool,
            tc.tile_pool(name="acc", bufs=3) as acc_pool,
            tc.tile_pool(name="pair", bufs=3, space="PSUM") as pair_ps,
            tc.tile_pool(name="gtw", bufs=1, space="PSUM") as gtw_ps,
            tc.tile_pool(name="gbk", bufs=1, space="PSUM") as gback_ps,
            tc.tile_pool(name="bps", bufs=1, space="PSUM") as b_ps,
            tc.tile_pool(name="wps", bufs=1, space="PSUM") as w_ps,
        ):
            ident = cpool.tile([128, 128], f32)
            make_identity(nc, ident)
            gw_sb = cpool.tile([128, KC * E], f32)
            gb_sb = cpool.tile([128, NT * E], f32)
            eb_sb = cpool.tile([E, DOUT], f32)
            eb_r = cpool.tile([E, DOUT], f32r)
            ew_r = cpool.tile([128, KC * E * DOUT], f32r)
            dma.dma_start(out=gw_sb, in_=gw_d[:, :])
            dma.dma_start(out=gb_sb, in_=gb_d[:, :])
            dma.dma_start(out=eb_sb, in_=eb_d[:, :])
            nc.vector.tensor_copy(eb_r, eb_sb)

            with tc.tile_pool(name="stage", bufs=1) as stage:
                ew_st = stage.tile([128, KC * E * DOUT], f32)
                dma.dma_start(out=ew_st, in_=ew_d[:, :])
                # round fp32 -> float32r for the TensorE fast path
                if use_act_round:
                    nc.scalar.copy(out=ew_r, in_=ew_st)
                else:
                    nc.vector.tensor_copy(ew_r, ew_st)

            def one_pass():
                for g in range(NG):
                    xg = xg_pool.tile([128, KC * TG], f32, tag="xg")
                    dma.dma_start(
                        out=xg.rearrange("p (k c) -> p k c", k=KC),
                        in_=xT_d.rearrange("(k p) t -> p k t", k=KC, p=128)
                        [:, :, g * TG:(g + 1) * TG],
                    )
                    xgr = xgr_pool.tile([128, KC * TG], f32r, tag="xgr")
                    if use_act_round:
                        nc.scalar.copy(out=xgr, in_=xg)
                    else:
                        nc.vector.tensor_copy(xgr, xg)

                    wm_g = wm_pool.tile([128, NT * E], f32, tag="wmg")
                    wps = w_ps.tile([8, NT * 128], f32, tag="wps")
                    # ---- gate, transposed: lgT[8, 512] in exact fp32 ----
                    gtp = gtw_ps.tile([8, TG], f32, tag="gtw")
                    for k in range(KC):
                        nc.tensor.matmul(
                            gtp,
                            gw_sb[:, k * E:(k + 1) * E],
                            xg[:, k * TG:(k + 1) * TG],
                            start=(k == 0), stop=(k == KC - 1),
                        )
                    lgT = sm.tile([8, TG], f32, tag="lgT")
                    nc.scalar.copy(out=lgT, in_=gtp)
                    # transpose back to [128 tok, 8] per tile
                    gbk = gback_ps.tile([128, NT * E], f32, tag="gbk")
                    for t in range(NT):
                        nc.tensor.transpose(
                            gbk[:, t * E:(t + 1) * E],
                            lgT[:, t * 128:(t + 1) * 128], ident[:8, :8])
                    lg_g = sm.tile([128, NT * E], f32, tag="lg")
                    nc.vector.tensor_add(lg_g, gbk, gb_sb)
                    ssum_g = sm.tile([128, NT], f32, tag="ssum")
                    rs_g = sm.tile([128, NT], f32, tag="rs")
                    for t in range(NT):
                        lg = lg_g[:, t * E:(t + 1) * E]
                        # ---- softmax + top-2 mask ----
                        m8 = sm.tile([128, 8], f32, tag="m8")
                        nc.vector.max(out=m8, in_=lg)
                        nm1 = sm.tile([128, 1], f32, tag="nm1")
                        nc.vector.tensor_scalar_mul(nm1, m8[:, 0:1], -1.0)
                        keep = sm.tile([128, E], f32, tag="keep")
                        nc.vector.tensor_scalar(
                            keep, lg, m8[:, 1:2], scalar2=None, op0=AL.is_ge)
                        texp = sm.tile([128, E], f32, tag="texp")
                        nc.scalar.activation(
                            texp, lg, AF.Exp, bias=nm1[:, 0:1], scale=1.0,
                            accum_out=ssum_g[:, t:t + 1])
                        # wm_pre = texp * keep (normalize after, batched)
                        nc.vector.tensor_mul(
                            wm_g[:, t * E:(t + 1) * E], texp, keep)
                    nc.vector.reciprocal(rs_g, ssum_g)
                    for t in range(NT):
                        # wm = wm_pre / s
                        nc.vector.tensor_scalar(
                            wm_g[:, t * E:(t + 1) * E],
                            wm_g[:, t * E:(t + 1) * E],
                            rs_g[:, t:t + 1], scalar2=None, op0=AL.mult)
                        # wm^T for the expert-bias matmul
                        nc.tensor.transpose(
                            wps[:, t * 128:(t + 1) * 128],
                            wm_g[:, t * E:(t + 1) * E], ident)

                    wmT_r = wmt_pool.tile([8, NT * 128], f32r, tag="wmt")
                    nc.vector.tensor_copy(wmT_r, wps)

                    bp = b_ps.tile([128, NT * DOUT], f32, tag="bp")
                    for t in range(NT):
                        nc.tensor.matmul(
                            bp[:, t * DOUT:(t + 1) * DOUT],
                            wmT_r[:, t * 128:(t + 1) * 128],
                            eb_r, start=True, stop=True)
                    acc_g = acc_pool.tile([128, NT * DOUT], f32, tag="acc")
                    if True:
                        for t in range(NT):
                            acc = acc_g[:, t * DOUT:(t + 1) * DOUT]
                            for pr in range(4):
                                pp = pair_ps.tile([128, 2 * DOUT], f32,
                                                  tag="pp", name=f"pp{pr}")
                                for k in range(KC):
                                    nc.tensor.matmul(
                                        pp,
                                        xgr[:, k * TG + t * 128: k * TG + (t + 1) * 128],
                                        ew_r[:, k * E * DOUT + 2 * pr * DOUT:
                                             k * E * DOUT + (2 * pr + 2) * DOUT],
                                        start=(k == 0), stop=(k == KC - 1),
                                    )
                                w0 = wm_g[:, t * E + 2 * pr: t * E + 2 * pr + 1]
                                w1 = wm_g[:, t * E + 2 * pr + 1: t * E + 2 * pr + 2]
                                if pr == 0:
                                    nc.vector.tensor_scalar(
                                        acc, pp[:, 0:DOUT], w0, scalar2=None,
                                        op0=AL.mult)
                                else:
                                    nc.vector.scalar_tensor_tensor(
                                        out=acc, in0=pp[:, 0:DOUT], scalar=w0,
                                        in1=acc, op0=AL.mult, op1=AL.add)
                                nc.vector.scalar_tensor_tensor(
                                    out=acc, in0=pp[:, DOUT:2 * DOUT], scalar=w1,
                                    in1=acc, op0=AL.mult, op1=AL.add)
                        nc.vector.tensor_add(acc_g, acc_g, bp)
                    dma.dma_start(
                        out=out_d.rearrange("(gg t p) n -> p (gg t) n", p=128, t=NT)
                        [:, g * NT:(g + 1) * NT, :],
                        in_=acc_g.rearrange("p (t n) -> p t n", t=NT),
                    )

            if reps == 1:
                one_pass()
            else:
                with tc.For_i(0, reps, 1):
                    one_pass()

    nc.compile()
    return nc


def _host_prep_weights(gate_W, gate_b, expert_W, expert_b):
    """Rearrange weights into the DMA-friendly layouts (replicated per core)."""
    gate_W = np.asarray(gate_W, dtype=np.float32)
    gate_b = np.asarray(gate_b, dtype=np.float32)
    expert_W = np.asarray(expert_W, dtype=np.float32)
    expert_b = np.asarray(expert_b, dtype=np.float32)
    # gw[p, k*8+j] = gate_W[k*128+p, j]
    gw = np.ascontiguousarray(
        gate_W.reshape(KC, 128, E).transpose(1, 0, 2).reshape(128, KC * E))
    gb = np.ascontiguousarray(np.tile(gate_b[None, :], (128, NT)))
    # ew[p, k*2048 + e*256 + n] = expert_W[e, k*128+p, n]
    ew = np.ascontiguousarray(
        expert_W.reshape(E, KC, 128, DOUT).transpose(2, 1, 0, 3)
        .reshape(128, KC * E * DOUT))
    eb = np.ascontiguousarray(expert_b)
    return gw, gb, ew, eb


def _get_runner(reps: int = 1, **build_kwargs):
    key = ("runner", reps, tuple(sorted(build_kwargs.items())))
    if key in _STATE:
        return _STATE[key]

    import jax
    from jax.sharding import Mesh, PartitionSpec
    from jax.experimental.shard_map import shard_map
    import concourse.mybir as mybir
    from concourse.bass2jax import (
        _bass_exec_p, install_neuronx_cc_hook, partition_id_tensor)

    nc = _build_program(reps=reps, **build_kwargs)
    install_neuronx_cc_hook()

    partition_name = (nc.partition_id_tensor.name
                      if nc.partition_id_tensor else None)
    in_names, out_names, out_avals = [], [], []
    for alloc in nc.m.functions[0].allocations:
        if not isinstance(alloc, mybir.MemoryLocationSet):
            continue
        name = alloc.memorylocations[0].name
        if alloc.kind == "ExternalInput":
            if name != partition_name:
                in_names.append(name)
        elif alloc.kind == "ExternalOutput":
            out_names.append(name)
            out_avals.append(jax.core.ShapedArray(
                tuple(alloc.tensor_shape), mybir.dt.np(alloc.dtype)))
    all_in_names = tuple(in_names) + tuple(out_names)
    if partition_name is not None:
        all_in_names = all_in_names + (partition_name,)
    n_params = len(in_names)

    def _body(*args):
        operands = list(args)
        if partition_name is not None:
            operands.append(partition_id_tensor())
        outs = _bass_exec_p.bind(
            *operands,
            out_avals=tuple(out_avals),
            in_names=all_in_names,
            out_names=tuple(out_names),
            lowering_input_output_aliases=(),
            sim_require_finite=True,
            sim_require_nnan=True,
            nc=nc,
        )
        return tuple(outs)

    devices = jax.devices()[:NCORES]
    mesh = Mesh(np.asarray(devices), ("core",))
    P = PartitionSpec("core")
    n_outs = len(out_names)
    fn = jax.jit(
        shard_map(_body, mesh=mesh,
                  in_specs=(P,) * (n_params + n_outs),
                  out_specs=(P,) * n_outs, check_rep=False),
        donate_argnums=tuple(range(n_params, n_params + n_outs)),
        keep_unused=True,
    )
    runner = {
        "nc": nc, "fn": fn, "in_names": in_names, "out_names": out_names,
        "out_avals": out_avals, "mesh": mesh,
    }
    _STATE[key] = runner
    return runner


def _make_concat_inputs(x, gate_W, gate_b, expert_W, expert_b):
    """Build the concatenated (8*dim0, ...) input arrays in in_names order."""
    x = np.asarray(x, dtype=np.float32)
    gw, gb, ew, eb = _host_prep_weights(gate_W, gate_b, expert_W, expert_b)
    toks = x.reshape(NTOK, DIN)
    # per-core transposed shards, stacked: xt_cat[c*DIN:(c+1)*DIN] = shard_c.T
    xt_cat = np.empty((NCORES * DIN, T), np.float32)
    for c in range(NCORES):
        xt_cat[c * DIN:(c + 1) * DIN] = toks[c * T:(c + 1) * T].T
    reps = {
        "xt": xt_cat,
        "gw": np.concatenate([gw] * NCORES, axis=0),
        "gb": np.concatenate([gb] * NCORES, axis=0),
        "ew": np.concatenate([ew] * NCORES, axis=0),
        "eb": np.concatenate([eb] * NCORES, axis=0),
    }
    return reps


def kernel(x, gate_W, gate_b, expert_W, expert_b):
    runner = _get_runner(reps=1)
    cat = _make_concat_inputs(x, gate_W, gate_b, expert_W, expert_b)
    concat_in = [cat[nm] for nm in runner["in_names"]]
    zeros = [np.zeros((NCORES * a.shape[0], *a.shape[1:]), a.dtype)
             for a in runner["out_avals"]]
    outs = runner["fn"](*concat_in, *zeros)
    out_cat = np.asarray(outs[runner["out_names"].index("out")])
    return out_cat.reshape(NCORES * T, DOUT).reshape(BS, L, DOUT)



# revision 4
# speedup vs baseline: 2.8389x; 2.8389x over previous
"""MoE text projection kernel for 8 TRN2 NeuronCores (Bass/Tile).

Problem: x[32,1024,768], gate_W[768,8], gate_b[8], expert_W[8,768,256],
expert_b[8,256] -> out[32,1024,256].  top-2 of 8 experts, softmax-over-all
gate, dense all-expert projection with masked weighted combine.

Strategy (v2): data-parallel over tokens (32768 -> 4096/core), all-f16
matmul operands (fp16 keeps a 10-bit mantissa; on the fixed test input this
flips 12/32768 top-2 selections, end-to-end rel err ~1.2e-2 < 2e-2 budget).

Per core, per 128-token tile:
  - 6 contraction chunks; per chunk ONE f16 stationary load of the x-tile,
    shared by 4 expert-pair matmuls (N=512, 1 cyc/row) AND a tiny N=8 gate
    matmul.  This replaces the baseline's separate fp32 gate pass (fp32
    matmuls are 4 cyc/row = ~903ns/MM measured vs 279ns for f16) and all
    its PSUM transposes.
  - expert biases folded into the PSUM accumulation via K=1 ones-row
    matmuls (moving operand = expert_b pairs) - no separate bias matmul
    or wm transpose needed.
  - softmax + top-2 mask on VectorE in fp32; weighted combine via
    per-partition-scalar fused multiply-add chains; output stored f16.
Host pre-transposes x to f16 xT[768, 4096] per core and upcasts the f16
output back to fp32.  No collectives: disjoint token shards.
"""
import sys

sys.path.insert(0, "/opt/trn_rl_repo")

import numpy as np

# hardcoded problem shapes
BS, L, DIN, DOUT, E = 32, 1024, 768, 256, 8
NCORES = 8
NTOK = BS * L              # 32768
T = NTOK // NCORES         # 4096 tokens per core
KC = DIN // 128            # 6 contraction chunks
NG = 8                     # groups per core
TG = T // NG               # 512 tokens per group
NT = TG // 128             # 4 tiles per group

_STATE: dict = {}


def _build_program(reps: int = 1):
    import concourse.mybir as mybir
    from concourse import bacc
    from concourse.tile import TileContext

    f32 = mybir.dt.float32
    f16 = mybir.dt.float16

    nc = bacc.Bacc("TRN2", target_bir_lowering=False, debug=False,
                   num_devices=NCORES)
    xT_d = nc.dram_tensor("xt", [DIN, T], f16, kind="ExternalInput")
    gw_d = nc.dram_tensor("gw", [128, KC * E], f16, kind="ExternalInput")
    gb_d = nc.dram_tensor("gb", [128, E], f32, kind="ExternalInput")
    ew_d = nc.dram_tensor("ew", [128, KC * E * DOUT], f16, kind="ExternalInput")
    eb_d = nc.dram_tensor("eb", [1, E * DOUT], f16, kind="ExternalInput")
    out_d = nc.dram_tensor("out", [T, DOUT], f16, kind="ExternalOutput")

    AL = mybir.AluOpType
    AF = mybir.ActivationFunctionType

    with TileContext(nc) as tc:
        with (
            tc.tile_pool(name="const", bufs=1) as cpool,
            tc.tile_pool(name="xg", bufs=2) as xg_pool,
            tc.tile_pool(name="sm", bufs=4) as sm,
            tc.tile_pool(name="accf", bufs=3) as accf_pool,
            tc.tile_pool(name="acc", bufs=2) as acc_pool,
            tc.tile_pool(name="pph", bufs=3, space="PSUM") as pp_ps,
            tc.tile_pool(name="gps", bufs=2, space="PSUM") as g_ps,
        ):
            gw_sb = cpool.tile([128, KC * E], f16)
            gb_sb = cpool.tile([128, E], f32)
            eb_sb = cpool.tile([1, E * DOUT], f16)
            ew_sb = cpool.tile([128, KC * E * DOUT], f16)
            ones_sb = cpool.tile([1, 128], f16)
            nc.sync.dma_start(out=gw_sb, in_=gw_d[:, :])
            nc.sync.dma_start(out=gb_sb, in_=gb_d[:, :])
            nc.sync.dma_start(out=eb_sb, in_=eb_d[:, :])
            nc.sync.dma_start(out=ew_sb, in_=ew_d[:, :])
            nc.vector.memset(ones_sb, 1.0)

            def one_tile(xg, t, acc_g):
                xt = [xg[:, k * TG + t * 128: k * TG + (t + 1) * 128]
                      for k in range(KC)]
                gps = g_ps.tile([128, E], f32, tag="gps")
                pps = []
                for h in range(2):
                    pp = pp_ps.tile([128, 2 * 512], f32, tag="pp",
                                    name=f"pp{h}")
                    pps.append(pp)
                    for k in range(KC):
                        for pr in range(2):
                            e0 = h * 4 + pr * 2
                            nc.tensor.matmul(
                                pp[:, pr * 512:(pr + 1) * 512],
                                xt[k],
                                ew_sb[:, (k * E + e0) * DOUT:
                                      (k * E + e0 + 2) * DOUT],
                                start=(k == 0), stop=False,
                            )
                        if h == 0:
                            # gate matmul shares the stationary x-tile
                            nc.tensor.matmul(
                                gps, xt[k], gw_sb[:, k * E:(k + 1) * E],
                                start=(k == 0), stop=(k == KC - 1),
                            )
                    # expert bias via K=1 ones-row matmuls (closes accum)
                    for pr in range(2):
                        e0 = h * 4 + pr * 2
                        nc.tensor.matmul(
                            pp[:, pr * 512:(pr + 1) * 512],
                            ones_sb[:, 0:128],
                            eb_sb[:, e0 * DOUT:(e0 + 2) * DOUT],
                            start=False, stop=True,
                        )

                # ---- gate softmax + top-2 mask (fp32) ----
                lg = sm.tile([128, E], f32, tag="lg")
                nc.vector.tensor_add(lg, gps, gb_sb)
                m8 = sm.tile([128, 8], f32, tag="m8")
                nc.vector.max(out=m8, in_=lg)
                nm1 = sm.tile([128, 1], f32, tag="nm1")
                nc.vector.tensor_scalar_mul(nm1, m8[:, 0:1], -1.0)
                keep = sm.tile([128, E], f32, tag="keep")
                nc.vector.tensor_scalar(
                    keep, lg, m8[:, 1:2], scalar2=None, op0=AL.is_ge)
                ssum = sm.tile([128, 1], f32, tag="ssum")
                texp = sm.tile([128, E], f32, tag="texp")
                nc.scalar.activation(
                    texp, lg, AF.Exp, bias=nm1[:, 0:1], scale=1.0,
                    accum_out=ssum)
                wmp = sm.tile([128, E], f32, tag="wmp")
                nc.vector.tensor_mul(wmp, texp, keep)
                rs = sm.tile([128, 1], f32, tag="rs")
                nc.vector.reciprocal(rs, ssum)
                wm = sm.tile([128, E], f32, tag="wm")
                nc.vector.tensor_scalar(
                    wm, wmp, rs, scalar2=None, op0=AL.mult)

                # ---- weighted combine: acc = sum_e wm[:,e] * pp_e ----
                accf = accf_pool.tile([128, DOUT], f32, tag="accf")
                nc.vector.tensor_scalar(
                    accf, pps[0][:, 0:DOUT], wm[:, 0:1],
                    scalar2=None, op0=AL.mult)
                for e in range(1, E):
                    src = pps[e // 4][:, (e % 4) * DOUT:(e % 4 + 1) * DOUT]
                    dst = accf if e < E - 1 else acc_g[:, t * DOUT:(t + 1) * DOUT]
                    nc.vector.scalar_tensor_tensor(
                        out=dst, in0=src, scalar=wm[:, e:e + 1],
                        in1=accf, op0=AL.mult, op1=AL.add)

            def one_pass():
                for g in range(NG):
                    xg = xg_pool.tile([128, KC * TG], f16, tag="xg")
                    nc.sync.dma_start(
                        out=xg.rearrange("p (k c) -> p k c", k=KC),
                        in_=xT_d.rearrange("(k p) t -> p k t", k=KC, p=128)
                        [:, :, g * TG:(g + 1) * TG],
                    )
                    acc_g = acc_pool.tile([128, NT * DOUT], f16, tag="acc")
                    for t in range(NT):
                        one_tile(xg, t, acc_g)
                    nc.sync.dma_start(
                        out=out_d.rearrange("(gg t p) n -> p (gg t) n",
                                            p=128, t=NT)
                        [:, g * NT:(g + 1) * NT, :],
                        in_=acc_g.rearrange("p (t n) -> p t n", t=NT),
                    )

            if reps == 1:
                one_pass()
            else:
                with tc.For_i(0, reps, 1):
                    one_pass()

    nc.compile()
    return nc


def _host_prep(gate_W, gate_b, expert_W, expert_b):
    """Rearrange weights/activations into DMA-friendly f16 layouts."""
    gate_W = np.asarray(gate_W, dtype=np.float32)
    gate_b = np.asarray(gate_b, dtype=np.float32)
    expert_W = np.asarray(expert_W, dtype=np.float32)
    expert_b = np.asarray(expert_b, dtype=np.float32)
    # gw[p, k*8+e] = gate_W[k*128+p, e]
    gw = np.ascontiguousarray(
        gate_W.reshape(KC, 128, E).transpose(1, 0, 2)
        .reshape(128, KC * E)).astype(np.float16)
    gb = np.ascontiguousarray(np.tile(gate_b[None, :], (128, 1)))
    # ew[p, (k*8+e)*256+n] = expert_W[e, k*128+p, n]
    ew = np.ascontiguousarray(
        expert_W.reshape(E, KC, 128, DOUT).transpose(2, 1, 0, 3)
        .reshape(128, KC * E * DOUT)).astype(np.float16)
    eb = np.ascontiguousarray(expert_b.reshape(1, E * DOUT)).astype(np.float16)
    return gw, gb, ew, eb


def _get_runner(reps: int = 1):
    key = ("runner", reps)
    if key in _STATE:
        return _STATE[key]

    import jax
    from jax.sharding import Mesh, PartitionSpec
    from jax.experimental.shard_map import shard_map
    import concourse.mybir as mybir
    from concourse.bass2jax import (
        _bass_exec_p, install_neuronx_cc_hook, partition_id_tensor)

    nc = _build_program(reps=reps)
    install_neuronx_cc_hook()

    partition_name = (nc.partition_id_tensor.name
                      if nc.partition_id_tensor else None)
    in_names, out_names, out_avals = [], [], []
    for alloc in nc.m.functions[0].allocations:
        if not isinstance(alloc, mybir.MemoryLocationSet):
            continue
        name = alloc.memorylocations[0].name
        if alloc.kind == "ExternalInput":
            if name != partition_name:
                in_names.append(name)
        elif alloc.kind == "ExternalOutput":
            out_names.append(name)
            out_avals.append(jax.core.ShapedArray(
                tuple(alloc.tensor_shape), mybir.dt.np(alloc.dtype)))
    all_in_names = tuple(in_names) + tuple(out_names)
    if partition_name is not None:
        all_in_names = all_in_names + (partition_name,)
    n_params = len(in_names)

    def _body(*args):
        operands = list(args)
        if partition_name is not None:
            operands.append(partition_id_tensor())
        outs = _bass_exec_p.bind(
            *operands,
            out_avals=tuple(out_avals),
            in_names=all_in_names,
            out_names=tuple(out_names),
            lowering_input_output_aliases=(),
            sim_require_finite=True,
            sim_require_nnan=True,
            nc=nc,
        )
        return tuple(outs)

    devices = jax.devices()[:NCORES]
    mesh = Mesh(np.asarray(devices), ("core",))
    P = PartitionSpec("core")
    n_outs = len(out_names)
    fn = jax.jit(
        shard_map(_body, mesh=mesh,
                  in_specs=(P,) * (n_params + n_outs),
                  out_specs=(P,) * n_outs, check_rep=False),
        donate_argnums=tuple(range(n_params, n_params + n_outs)),
        keep_unused=True,
    )
    runner = {
        "nc": nc, "fn": fn, "in_names": in_names, "out_names": out_names,
        "out_avals": out_avals, "mesh": mesh,
    }
    _STATE[key] = runner
    return runner


def _make_concat_inputs(x, gate_W, gate_b, expert_W, expert_b):
    """Build the concatenated (8*dim0, ...) input arrays in in_names order."""
    x = np.asarray(x, dtype=np.float32)
    gw, gb, ew, eb = _host_prep(gate_W, gate_b, expert_W, expert_b)
    toks = x.reshape(NTOK, DIN)
    # per-core transposed f16 shards, stacked
    xt_cat = np.empty((NCORES * DIN, T), np.float16)
    for c in range(NCORES):
        xt_cat[c * DIN:(c + 1) * DIN] = toks[c * T:(c + 1) * T].T
    reps = {
        "xt": xt_cat,
        "gw": np.concatenate([gw] * NCORES, axis=0),
        "gb": np.concatenate([gb] * NCORES, axis=0),
        "ew": np.concatenate([ew] * NCORES, axis=0),
        "eb": np.concatenate([eb] * NCORES, axis=0),
    }
    return reps


def kernel(x, gate_W, gate_b, expert_W, expert_b):
    runner = _get_runner(reps=1)
    cat = _make_concat_inputs(x, gate_W, gate_b, expert_W, expert_b)
    concat_in = [cat[nm] for nm in runner["in_names"]]
    zeros = [np.zeros((NCORES * a.shape[0], *a.shape[1:]), a.dtype)
             for a in runner["out_avals"]]
    outs = runner["fn"](*concat_in, *zeros)
    out_cat = np.asarray(outs[runner["out_names"].index("out")])
    return out_cat.reshape(NCORES * T, DOUT).astype(np.float32).reshape(
        BS, L, DOUT)


# revision 6
# speedup vs baseline: 7.4575x; 2.6268x over previous
"""MoE text projection kernel for 8 TRN2 NeuronCores (Bass/Tile).

Problem: x[32,1024,768], gate_W[768,8], gate_b[8], expert_W[8,768,256],
expert_b[8,256] -> out[32,1024,256].  top-2 of 8 experts, softmax-over-all
gate, dense all-expert projection with masked weighted combine.

Strategy (v2): data-parallel over tokens (32768 -> 4096/core), all-f16
matmul operands (fp16 keeps a 10-bit mantissa; on the fixed test input this
flips 12/32768 top-2 selections, end-to-end rel err ~1.2e-2 < 2e-2 budget).

Per core, per 128-token tile:
  - 6 contraction chunks; per chunk ONE f16 stationary load of the x-tile,
    shared by 4 expert-pair matmuls (N=512, 1 cyc/row) AND a tiny N=8 gate
    matmul.  This replaces the baseline's separate fp32 gate pass (fp32
    matmuls are 4 cyc/row = ~903ns/MM measured vs 279ns for f16) and all
    its PSUM transposes.
  - expert biases folded into the PSUM accumulation via K=1 ones-row
    matmuls (moving operand = expert_b pairs) - no separate bias matmul
    or wm transpose needed.
  - softmax + top-2 mask on VectorE in fp32; weighted combine via
    per-partition-scalar fused multiply-add chains; output stored f16.
Host pre-transposes x to f16 xT[768, 4096] per core and upcasts the f16
output back to fp32.  No collectives: disjoint token shards.
"""
import sys

sys.path.insert(0, "/opt/trn_rl_repo")

import numpy as np

# hardcoded problem shapes
BS, L, DIN, DOUT, E = 32, 1024, 768, 256, 8
NCORES = 8
NTOK = BS * L              # 32768
T = NTOK // NCORES         # 4096 tokens per core
KC = DIN // 128            # 6 contraction chunks
NG = 8                     # groups per core
TG = T // NG               # 512 tokens per group
NT = TG // 128             # 4 tiles per group

_STATE: dict = {}


def _build_program(reps: int = 1):
    import concourse.mybir as mybir
    from concourse import bacc
    from concourse.tile import TileContext

    f32 = mybir.dt.float32
    f16 = mybir.dt.float16

    nc = bacc.Bacc("TRN2", target_bir_lowering=False, debug=False,
                   num_devices=NCORES)
    xT_d = nc.dram_tensor("xt", [DIN, T], f16, kind="ExternalInput")
    gw_d = nc.dram_tensor("gw", [128, KC * E], f16, kind="ExternalInput")
    gb_d = nc.dram_tensor("gb", [128, E], f32, kind="ExternalInput")
    ew_d = nc.dram_tensor("ew", [128, KC * E * DOUT], f16, kind="ExternalInput")
    eb_d = nc.dram_tensor("eb", [1, E * DOUT], f16, kind="ExternalInput")
    out_d = nc.dram_tensor("out", [T, DOUT], f16, kind="ExternalOutput")

    AL = mybir.AluOpType
    AF = mybir.ActivationFunctionType

    with TileContext(nc) as tc:
        with (
            tc.tile_pool(name="const", bufs=1) as cpool,
            tc.tile_pool(name="xg", bufs=2) as xg_pool,
            tc.tile_pool(name="sm", bufs=4) as sm,
            tc.tile_pool(name="accf", bufs=3) as accf_pool,
            tc.tile_pool(name="acc", bufs=2) as acc_pool,
            tc.tile_pool(name="pph", bufs=3, space="PSUM") as pp_ps,
            tc.tile_pool(name="gps", bufs=2, space="PSUM") as g_ps,
        ):
            gw_sb = cpool.tile([128, KC * E], f16)
            gb_sb = cpool.tile([128, E], f32)
            eb_sb = cpool.tile([1, E * DOUT], f16)
            ew_sb = cpool.tile([128, KC * E * DOUT], f16)
            ones_sb = cpool.tile([1, 128], f16)
            nc.sync.dma_start(out=gw_sb, in_=gw_d[:, :])
            nc.sync.dma_start(out=gb_sb, in_=gb_d[:, :])
            nc.sync.dma_start(out=eb_sb, in_=eb_d[:, :])
            nc.sync.dma_start(out=ew_sb, in_=ew_d[:, :])
            nc.vector.memset(ones_sb, 1.0)

            def one_tile(xg, t, acc_g):
                xt = [xg[:, k * TG + t * 128: k * TG + (t + 1) * 128]
                      for k in range(KC)]
                gps = g_ps.tile([128, E], f32, tag="gps")
                pps = [pp_ps.tile([128, 2 * 512], f32, tag="pp",
                                  name=f"pp{h}") for h in range(2)]
                # k-outer: one stationary load per chunk feeds 4 expert-pair
                # matmuls (4-bank PSUM interleave) + the tiny gate matmul
                for k in range(KC):
                    for h in range(2):
                        for pr in range(2):
                            e0 = h * 4 + pr * 2
                            nc.tensor.matmul(
                                pps[h][:, pr * 512:(pr + 1) * 512],
                                xt[k],
                                ew_sb[:, (k * E + e0) * DOUT:
                                      (k * E + e0 + 2) * DOUT],
                                start=(k == 0), stop=False,
                            )
                    nc.tensor.matmul(
                        gps, xt[k], gw_sb[:, k * E:(k + 1) * E],
                        start=(k == 0), stop=(k == KC - 1),
                    )
                # expert bias via K=1 ones-row matmuls (one stationary load,
                # closes all 4 accumulation groups)
                for h in range(2):
                    for pr in range(2):
                        e0 = h * 4 + pr * 2
                        nc.tensor.matmul(
                            pps[h][:, pr * 512:(pr + 1) * 512],
                            ones_sb[:, 0:128],
                            eb_sb[:, e0 * DOUT:(e0 + 2) * DOUT],
                            start=False, stop=True,
                        )

                # ---- gate softmax + top-2 mask (fp32, split DVE/Act) ----
                lg = sm.tile([128, E], f32, tag="lg")
                nc.vector.tensor_add(lg, gps, gb_sb)
                m8 = sm.tile([128, 8], f32, tag="m8")
                nc.vector.max(out=m8, in_=lg)
                nm1 = sm.tile([128, 1], f32, tag="nm1")
                nc.scalar.activation(nm1, m8[:, 0:1], AF.Copy, scale=-1.0)
                keep = sm.tile([128, E], f32, tag="keep")
                nc.vector.tensor_scalar(
                    keep, lg, m8[:, 1:2], scalar2=None, op0=AL.is_ge)
                ssum = sm.tile([128, 1], f32, tag="ssum")
                texp = sm.tile([128, E], f32, tag="texp")
                nc.scalar.activation(
                    texp, lg, AF.Exp, bias=nm1[:, 0:1], scale=1.0,
                    accum_out=ssum)
                wmp = sm.tile([128, E], f32, tag="wmp")
                nc.vector.tensor_mul(wmp, texp, keep)
                rs = sm.tile([128, 1], f32, tag="rs")
                nc.vector.reciprocal(rs, ssum)
                wm = sm.tile([128, E], f32, tag="wm")
                nc.vector.tensor_scalar(
                    wm, wmp, rs, scalar2=None, op0=AL.mult)

                # ---- weighted combine: acc = sum_e wm[:,e] * pp_e ----
                # odd experts scaled on ScalarE, even fused on VectorE
                aodd = [accf_pool.tile([128, DOUT], f32, tag="aodd",
                                       name=f"ao{j}") for j in range(4)]
                for j in range(4):
                    e = 2 * j + 1
                    nc.scalar.activation(
                        aodd[j], pps[e // 4][:, (e % 4) * DOUT:(e % 4 + 1) * DOUT],
                        AF.Copy, scale=wm[:, e:e + 1])
                sev = [accf_pool.tile([128, DOUT], f16, tag="sev",
                                      name=f"se{j}") for j in range(4)]
                for j in range(4):
                    e = 2 * j
                    nc.vector.scalar_tensor_tensor(
                        out=sev[j],
                        in0=pps[e // 4][:, (e % 4) * DOUT:(e % 4 + 1) * DOUT],
                        scalar=wm[:, e:e + 1], in1=aodd[j],
                        op0=AL.mult, op1=AL.add)
                u = accf_pool.tile([128, DOUT], f16, tag="u")
                v = accf_pool.tile([128, DOUT], f16, tag="v")
                nc.vector.tensor_add(u, sev[0], sev[1])
                nc.vector.tensor_add(v, sev[2], sev[3])
                nc.vector.tensor_add(acc_g[:, t * DOUT:(t + 1) * DOUT], u, v)

            def one_pass():
                for g in range(NG):
                    xg = xg_pool.tile([128, KC * TG], f16, tag="xg")
                    nc.sync.dma_start(
                        out=xg.rearrange("p (k c) -> p k c", k=KC),
                        in_=xT_d.rearrange("(k p) t -> p k t", k=KC, p=128)
                        [:, :, g * TG:(g + 1) * TG],
                    )
                    acc_g = acc_pool.tile([128, NT * DOUT], f16, tag="acc")
                    for t in range(NT):
                        one_tile(xg, t, acc_g)
                    nc.sync.dma_start(
                        out=out_d.rearrange("(gg t p) n -> p (gg t) n",
                                            p=128, t=NT)
                        [:, g * NT:(g + 1) * NT, :],
                        in_=acc_g.rearrange("p (t n) -> p t n", t=NT),
                    )

            if reps == 1:
                one_pass()
            else:
                with tc.For_i(0, reps, 1):
                    one_pass()

    nc.compile()
    return nc


def _host_prep(gate_W, gate_b, expert_W, expert_b):
    """Rearrange weights/activations into DMA-friendly f16 layouts."""
    gate_W = np.asarray(gate_W, dtype=np.float32)
    gate_b = np.asarray(gate_b, dtype=np.float32)
    expert_W = np.asarray(expert_W, dtype=np.float32)
    expert_b = np.asarray(expert_b, dtype=np.float32)
    # gw[p, k*8+e] = gate_W[k*128+p, e]
    gw = np.ascontiguousarray(
        gate_W.reshape(KC, 128, E).transpose(1, 0, 2)
        .reshape(128, KC * E)).astype(np.float16)
    gb = np.ascontiguousarray(np.tile(gate_b[None, :], (128, 1)))
    # ew[p, (k*8+e)*256+n] = expert_W[e, k*128+p, n]
    ew = np.ascontiguousarray(
        expert_W.reshape(E, KC, 128, DOUT).transpose(2, 1, 0, 3)
        .reshape(128, KC * E * DOUT)).astype(np.float16)
    eb = np.ascontiguousarray(expert_b.reshape(1, E * DOUT)).astype(np.float16)
    return gw, gb, ew, eb


def _get_runner(reps: int = 1):
    key = ("runner", reps)
    if key in _STATE:
        return _STATE[key]

    import jax
    from jax.sharding import Mesh, PartitionSpec
    from jax.experimental.shard_map import shard_map
    import concourse.mybir as mybir
    from concourse.bass2jax import (
        _bass_exec_p, install_neuronx_cc_hook, partition_id_tensor)

    nc = _build_program(reps=reps)
    install_neuronx_cc_hook()

    partition_name = (nc.partition_id_tensor.name
                      if nc.partition_id_tensor else None)
    in_names, out_names, out_avals = [], [], []
    for alloc in nc.m.functions[0].allocations:
        if not isinstance(alloc, mybir.MemoryLocationSet):
            continue
        name = alloc.memorylocations[0].name
        if alloc.kind == "ExternalInput":
            if name != partition_name:
                in_names.append(name)
        elif alloc.kind == "ExternalOutput":
            out_names.append(name)
            out_avals.append(jax.core.ShapedArray(
                tuple(alloc.tensor_shape), mybir.dt.np(alloc.dtype)))
    all_in_names = tuple(in_names) + tuple(out_names)
    if partition_name is not None:
        all_in_names = all_in_names + (partition_name,)
    n_params = len(in_names)

    def _body(*args):
        operands = list(args)
        if partition_name is not None:
            operands.append(partition_id_tensor())
        outs = _bass_exec_p.bind(
            *operands,
            out_avals=tuple(out_avals),
            in_names=all_in_names,
            out_names=tuple(out_names),
            lowering_input_output_aliases=(),
            sim_require_finite=True,
            sim_require_nnan=True,
            nc=nc,
        )
        return tuple(outs)

    devices = jax.devices()[:NCORES]
    mesh = Mesh(np.asarray(devices), ("core",))
    P = PartitionSpec("core")
    n_outs = len(out_names)
    fn = jax.jit(
        shard_map(_body, mesh=mesh,
                  in_specs=(P,) * (n_params + n_outs),
                  out_specs=(P,) * n_outs, check_rep=False),
        donate_argnums=tuple(range(n_params, n_params + n_outs)),
        keep_unused=True,
    )
    runner = {
        "nc": nc, "fn": fn, "in_names": in_names, "out_names": out_names,
        "out_avals": out_avals, "mesh": mesh,
    }
    _STATE[key] = runner
    return runner


def _make_concat_inputs(x, gate_W, gate_b, expert_W, expert_b):
    """Build the concatenated (8*dim0, ...) input arrays in in_names order."""
    x = np.asarray(x, dtype=np.float32)
    gw, gb, ew, eb = _host_prep(gate_W, gate_b, expert_W, expert_b)
    toks = x.reshape(NTOK, DIN)
    # per-core transposed f16 shards, stacked
    xt_cat = np.empty((NCORES * DIN, T), np.float16)
    for c in range(NCORES):
        xt_cat[c * DIN:(c + 1) * DIN] = toks[c * T:(c + 1) * T].T
    reps = {
        "xt": xt_cat,
        "gw": np.concatenate([gw] * NCORES, axis=0),
        "gb": np.concatenate([gb] * NCORES, axis=0),
        "ew": np.concatenate([ew] * NCORES, axis=0),
        "eb": np.concatenate([eb] * NCORES, axis=0),
    }
    return reps


def kernel(x, gate_W, gate_b, expert_W, expert_b):
    runner = _get_runner(reps=1)
    cat = _make_concat_inputs(x, gate_W, gate_b, expert_W, expert_b)
    concat_in = [cat[nm] for nm in runner["in_names"]]
    zeros = [np.zeros((NCORES * a.shape[0], *a.shape[1:]), a.dtype)
             for a in runner["out_avals"]]
    outs = runner["fn"](*concat_in, *zeros)
    out_cat = np.asarray(outs[runner["out_names"].index("out")])
    return out_cat.reshape(NCORES * T, DOUT).astype(np.float32).reshape(
        BS, L, DOUT)
